# revision 38
# baseline (speedup 1.0000x reference)
"""Trainium2 Bass kernel for the BackwardVariableSplitter pair scorer.

reference math:
    context = relu(nse @ Wc + bc)                      # [128]
    queries = pve @ Wq + bq + context                  # [1024, 128]
    keys    = pve @ Wk + bk + context                  # [1024, 128]
    q_proj  = queries @ W1[:128]                       # [1024, 128]
    k_proj  = keys @ W1[128:]                          # [1024, 128]
    hidden[i,j] = relu(q_proj[i] + k_proj[j] + b1)     # [1024, 1024, 128]
    scores[i,j] = hidden[i,j] @ W2 + b2                # [1024, 1024]
    out = scores[i, j] for i < j, row-major            # [523776]

The O(n*d*h) projections are tiny (<0.1% of FLOPs) and are done on the host;
the O(n^2*h) relu + weighted-reduce runs on 8 NeuronCores.

Sharding: core d owns query rows {i : i % 8 == d} (interleaved), so the
triangular (j > i) workload is balanced and the SPMD program is identical on
every core: local row k (global i = 8k + d) computes columns j in [8k, 1024).

Device program (per core):
  - X_k = relu(k_projT[:, 8k:] + qb_k)   [h=128 partitions, w=1024-8k free]
    one VectorE tensor_scalar (add, max0; fp16, 4x perf mode) or ScalarE
    activation per row, split between the two engines by measured cost
    (DVE ~218+0.254w ns, ACT ~280+0.833w ns). Rows are processed narrow
    first (matching kpt DMA chunk arrival), wide in the middle, mid rows
    last so the first eviction overlaps the final matmuls; the first and
    last few rows alternate engines so neither engine gates the ends.
  - TensorE: one-hot W2 stationary window [128, 32] puts w2 in column
    (k//4), so row k's scores accumulate into PSUM partition
    32*(k%4) + k//4 of bank jt = j//512. The k%4 rotation spreads
    consecutive matmuls across the 4 PE column groups, which execute
    concurrently (measured depth 3-6). Banks are pre-zeroed by start=True
    all-zero-weights matmuls so every real matmul is a pure accumulate.
  - evict psum -> sbuf (VectorE/ScalarE) -> DMA fp32 block out, chunked so
    the psA half streams out while psB is still accumulating.
"""

import os
import numpy as np

N = 1024
E = 256
H = 128
NCORES = 8
NROWS = N // NCORES  # 128 local rows per core

_PROG_CACHE = {}


def _row_width(k: int) -> int:
    # local row k computes columns [8k, 1024)
    return N - 8 * k


def _row_order():
    """Processing order: narrowest rows first (they only need the last kpt
    DMA chunk, which lands first), wide rows in the middle, and 16 mid rows
    last so the psA eviction + first output DMA overlap the final matmuls."""
    return (
        list(range(111, 95, -1))
        + list(range(64, 80))
        + list(range(0, 64))
        + list(range(80, 96))
        + list(range(112, 128))
    )


def _engine_assignment():
    """Static greedy split of the 128 row-instructions between VectorE and
    ScalarE, balancing measured cost (fits from HW trace):
      DVE tensor_scalar fp16 4x:  218 + 0.254*w ns
      ACT activation:             280 + 0.833*w ns
    ScalarE starts with a handicap for its DMA issue + psum evictions.
    """
    t_dve = 1650.0
    t_act = 4200.0
    assign = [None] * NROWS
    order = _row_order()
    # the first processed rows alternate so both engines start immediately,
    # and so do the last ones (ending on DVE) so the final X->matmul chain
    # isn't serialized on one engine
    for i, k in enumerate(order[:8]):
        assign[k] = "dve" if i % 2 == 0 else "act"
    # DVE-heavy tail (2:1), ending on DVE: its rows are ~2x cheaper, so the
    # final X->matmul chain drains through both engines without ACT gating it
    for i, k in enumerate(reversed(order[-12:])):
        assign[k] = "act" if i % 3 == 2 else "dve"
    for k in order[:8] + order[-12:]:
        w = _row_width(k)
        if assign[k] == "dve":
            t_dve += 218.0 + 0.254 * w
        else:
            t_act += 280.0 + 0.833 * w
    ks = sorted((k for k in range(NROWS) if assign[k] is None),
                key=lambda k: -_row_width(k))
    for k in ks:
        w = _row_width(k)
        c_dve = 218.0 + 0.254 * w
        c_act = 280.0 + 0.833 * w
        if t_dve + c_dve <= t_act + c_act:
            assign[k] = "dve"
            t_dve += c_dve
        else:
            assign[k] = "act"
            t_act += c_act
    return assign


def psum_partition(k: int) -> int:
    return 32 * (k % 4) + k // 4


def _build_program():
    """Build + schedule the single SPMD Bass program (shape-only, no data)."""
    import concourse.bacc as bacc
    import concourse.tile as tile
    import concourse.mybir as mybir

    nc = bacc.Bacc(
        "TRN2",
        target_bir_lowering=False,
        enable_partition_id=False,
        detect_race_conditions=False,
    )

    kpt_d = nc.dram_tensor("kpt", [H, N], mybir.dt.float16, kind="ExternalInput")
    qbt_d = nc.dram_tensor("qbt", [H, NROWS], mybir.dt.float32, kind="ExternalInput")
    # one-hot W2 window buffer: w2 in column 63, zeros elsewhere
    w2w_d = nc.dram_tensor("w2w", [H, 96], mybir.dt.float16, kind="ExternalInput")
    out_d = nc.dram_tensor("out", [H, N], mybir.dt.float32, kind="ExternalOutput")

    assign = _engine_assignment()
    order = _row_order()

    with tile.TileContext(nc) as tc:
        with (
            tc.tile_pool(name="const", bufs=1) as const,
            tc.tile_pool(name="xd", bufs=12) as xd_pool,
            tc.tile_pool(name="xa", bufs=12) as xa_pool,
            tc.tile_pool(name="ps", bufs=1, space="PSUM") as ps,
        ):
            kpt = const.tile([H, N], mybir.dt.float16)
            qbt = const.tile([H, NROWS], mybir.dt.float32)
            w2w = const.tile([H, 96], mybir.dt.float16)
            zw = const.tile([H, 256], mybir.dt.float16)  # zeros (stationary + rhs)
            out_sb = const.tile([H, N], mybir.dt.float32)

            # warm up ACT's Relu table before the DMAs land
            nc.scalar.activation(
                out_sb[:, 0:8],
                out_sb[:, 0:8],
                mybir.ActivationFunctionType.Relu,
            )

            # parallel DMA issue on separate queues; narrow rows (the first
            # processed) only need the tail chunk of kpt, which arrives first
            nc.sync.dma_start(kpt[:, 768:N], kpt_d[:, 768:N])
            nc.sync.dma_start(kpt[:, 512:768], kpt_d[:, 512:768])
            nc.scalar.dma_start(qbt[:], qbt_d[:])
            nc.scalar.dma_start(w2w[:], w2w_d[:])
            nc.scalar.dma_start(kpt[:, 0:512], kpt_d[:, 0:512])
            nc.vector.memset(zw[:], 0.0)

            psA = ps.tile([H, 512], mybir.dt.float32)  # columns j in [0, 512)
            psB = ps.tile([H, 512], mybir.dt.float32)  # columns j in [512, 1024)

            # pre-zero both banks (start=True sets has_written everywhere,
            # so every later matmul is a pure accumulate)
            for bank in (psA, psB):
                for half in range(2):
                    nc.tensor.matmul(
                        bank[:, 256 * half : 256 * half + 256],
                        zw[:, 0:H],
                        zw[:],
                        start=True,
                        stop=False,
                        skip_group_check=True,
                    )

            # the last processed row with j0 < 512 decides when psA is done
            last_jt0 = [k for k in order if 8 * k < 512][-1]
            last_psb = order[-1]

            for k in order:
                eng = assign[k]
                pool = xd_pool if eng == "dve" else xa_pool
                j0 = 8 * k
                w = N - j0
                xt = pool.tile([H, w], mybir.dt.float16, tag="x" + eng, name="x" + eng)
                x = xt[:, 0:w]
                if eng == "dve":
                    nc.vector.tensor_scalar(
                        x,
                        kpt[:, j0:N],
                        qbt[:, k : k + 1],
                        0.0,
                        op0=mybir.AluOpType.add,
                        op1=mybir.AluOpType.max,
                    )
                else:
                    nc.scalar.activation(
                        x,
                        kpt[:, j0:N],
                        mybir.ActivationFunctionType.Relu,
                        bias=qbt[:, k : k + 1],
                        scale=1.0,
                    )
                g = k % 4  # PE column group
                m = k // 4  # one-hot position within the 32-wide window
                lhsT = w2w[:, 63 - m : 95 - m]
                pslice = slice(32 * g, 32 * g + 32)
                if j0 < 512:
                    wa = 512 - j0
                    nc.tensor.matmul(
                        psA[pslice, j0:512],
                        lhsT,
                        x[:, 0:wa],
                        start=False,
                        stop=(k == last_jt0),
                        skip_group_check=True,
                        tile_position=(0, 32 * g),
                    )
                    nc.tensor.matmul(
                        psB[pslice, :],
                        lhsT,
                        x[:, wa : wa + 512],
                        start=False,
                        stop=(k == last_psb),
                        skip_group_check=True,
                        tile_position=(0, 32 * g),
                    )
                else:
                    nc.tensor.matmul(
                        psB[pslice, j0 - 512 : 512],
                        lhsT,
                        x[:],
                        start=False,
                        stop=(k == last_psb),
                        skip_group_check=True,
                        tile_position=(0, 32 * g),
                    )
                if k == last_jt0:
                    # bank jt0 complete: evict + store its half early
                    nc.vector.tensor_copy(out_sb[:, 0:512], psA[:])
                    nc.scalar.dma_start(out_d[:, 0:512], out_sb[:, 0:512])

            # single full-width eviction (Tile serializes same-bank readers
            # anyway), then the two output DMAs go out on both queues
            nc.vector.tensor_copy(out_sb[:, 512:N], psB[:])
            nc.scalar.dma_start(out_d[:, 512:768], out_sb[:, 512:768])
            nc.sync.dma_start(out_d[:, 768:N], out_sb[:, 768:N])

    nc.compile()
    return nc


def _get_program():
    if "nc" not in _PROG_CACHE:
        _PROG_CACHE["nc"] = _build_program()
    return _PROG_CACHE["nc"]


def _install_ntff_hook():
    """The agent image's ``antenv`` lacks ``axon_hooks``, so axon-side NTFF
    profiling silently degrades. Recreate the module and install the ctypes
    hook so trace=True yields exec_time_ns. No-op if unavailable."""
    import sys
    import types

    try:
        import antenv.axon_hooks  # noqa: F401

        return
    except ImportError:
        pass
    try:
        import antenv
        from trn_agent_boot.trn_boot import _ntff_profile_via_ctypes

        mod = types.ModuleType("antenv.axon_hooks")
        mod._hook = _ntff_profile_via_ctypes("/opt/axon/libaxon_pjrt.so")
        mod.set_axon_ntff_profile_hook = lambda h: setattr(mod, "_hook", h)
        mod.get_axon_ntff_profile_hook = lambda: mod._hook
        sys.modules["antenv.axon_hooks"] = mod
        antenv.axon_hooks = mod
    except Exception:
        pass


def kernel(
    next_state_embedding,
    prev_variable_embeddings,
    Wq,
    bq,
    Wk,
    bk,
    Wc,
    bc,
    W1,
    b1,
    W2,
    b2,
):
    from concourse.bass_utils import run_bass_kernel_spmd

    trace = bool(int(os.environ.get("KBENCH_TRACE", "0")))
    if trace:
        _install_ntff_hook()

    nse = np.asarray(next_state_embedding, dtype=np.float32)
    pve = np.asarray(prev_variable_embeddings, dtype=np.float32)
    Wq = np.asarray(Wq, dtype=np.float32)
    bq = np.asarray(bq, dtype=np.float32)
    Wk = np.asarray(Wk, dtype=np.float32)
    bk = np.asarray(bk, dtype=np.float32)
    Wc = np.asarray(Wc, dtype=np.float32)
    bc = np.asarray(bc, dtype=np.float32)
    W1 = np.asarray(W1, dtype=np.float32)
    b1 = np.asarray(b1, dtype=np.float32)
    W2 = np.asarray(W2, dtype=np.float32)
    b2 = np.asarray(b2, dtype=np.float32)

    # host-side projections (tiny)
    context = np.maximum(nse @ Wc + bc, 0.0)
    queries = pve @ Wq + bq + context
    keys = pve @ Wk + bk + context
    q_proj = queries @ W1[:H]  # [N, H]
    k_proj = keys @ W1[H:]  # [N, H]

    kpt = np.ascontiguousarray(k_proj.T, dtype=np.float16)  # [H, N]
    w2w = np.zeros((H, 96), dtype=np.float16)
    w2w[:, 63] = W2[:, 0].astype(np.float16)

    in_maps = []
    for d in range(NCORES):
        qb = q_proj[d::NCORES] + b1  # [128, H]
        qbt = np.ascontiguousarray(qb.T, dtype=np.float32)  # [H, 128]
        in_maps.append({"kpt": kpt, "qbt": qbt, "w2w": w2w})

    nc = _get_program()
    res = None
    for attempt in range(3):
        try:
            res = run_bass_kernel_spmd(
                nc,
                in_maps,
                core_ids=list(range(NCORES)),
                trace=trace,
            )
            break
        except Exception:
            if attempt == 2:
                raise
            import time

            time.sleep(2.0)
    kernel.last_results = res

    perm = np.array([psum_partition(k) for k in range(NROWS)])
    scores = np.empty((N, N), dtype=np.float32)
    for d in range(NCORES):
        scores[d::NCORES, :] = res.results[d]["out"][perm, :]

    iu, ju = np.triu_indices(N, k=1)
    return (scores[iu, ju] + b2[0]).astype(np.float32)


kernel.last_results = None
